# revision 11
# baseline (speedup 1.0000x reference)
"""Trainium2 Bass kernel for nn_Criterion_74448963109285 (segment_reduce criterion).

Strategy (pure data parallel, 2 images per core on 8 cores):
  Per image the loss is  intra + inter + ce  where every term reduces to a
  handful of tiny quantities:
    - segment sums over pixels per label l in {1,2}:
        t_l[e] = sum_{p: lab=l} emb[e,p]          (raw)
        s_l[e] = sum_{p: lab=l} emb[e,p]/||emb_p|| (normalized)
        c_l    = count of pixels with label l
    - ce partials: sum_p logsumexp(pred[:,p]) and sum_p pred[lab_p, p]
  The device computes only these reductions; the final scalar math runs on
  host in float64.

  Engine plan (per core = 2 images, 32 tiles of [128, 4096]):
    DMA    one 2MB dma_start per tile (HWDGE, alternating sync/scalar rings)
    DVE    stream-transpose f32->bf16 (cast fused into the transpose),
           level-1 halving add of the squares (scalar_tensor_tensor, 4x mode),
           CE picked-logit accumulation, label/weight prep
    Scalar SQUARE activation on the transposed tile, Rsqrt for 1/||x||,
           CE exp/ln (+ row-accumulated logsumexp)
    GpSimd levels 2-5 of the norm2 halving-add tree
    PE     block-diagonal matmuls accumulating {oh1,oh2,oh1/|x|,oh2/|x|} x
           {32 channels} segment sums into one PSUM tile
  Counts come for free from accum_out on the is_equal weight builds, so the
  matmul rhs has no ones column (stride-32, fully aligned).
"""

import numpy as np

import concourse.bass as bass
import concourse.tile as tile
from concourse import mybir
from concourse.bass_utils import run_bass_kernel_spmd

F32 = mybir.dt.float32
BF16 = mybir.dt.bfloat16
I32 = mybir.dt.int32
ALU = mybir.AluOpType
ACTF = mybir.ActivationFunctionType

B, E, H, W, L = 16, 32, 512, 512, 3
P = H * W                  # 262144 pixels per image
NCORES = 8
BLOC = B // NCORES         # 2 images per core
G = 4                      # pixel groups packed into partitions (4*32ch=128)
PG = P // G                # 65536 pixels per group
NT = 16                    # tiles per image
FCOLS = PG // NT           # 4096 pixel columns per tile (per group)
CB = FCOLS // 32           # 128 c-blocks (32 px each) per tile
CIMG = PG // 32            # 2048 c-blocks per image
TGRP = 2                   # tiles per rsqrt/weight batch group
CGRP = 8                   # c-blocks per matmul (N = 8*32 = 256)
MM_M = 4 * CGRP            # 32 output partitions
MM_N = 32 * CGRP           # 256 output cols
PCOLS = P // 128           # 2048 label/pred columns per image
NSC = 8                    # small-result columns


def _split_oversized_waits(nc, max_waits=1):
    """This walrus build accepts only one sync wait per instruction; move
    extra waits onto single-wait NOPs preceding the instruction."""
    for fn in nc.m.functions:
        for blk in fn.blocks:
            new_list = []
            for ins in blk.instructions:
                si = getattr(ins, "sync_info", None)
                if si is not None and si.on_wait and len(si.on_wait) > max_waits:
                    waits = list(si.on_wait)
                    chunks = [
                        waits[i : i + max_waits]
                        for i in range(0, len(waits), max_waits)
                    ]
                    for j, ch in enumerate(chunks[:-1]):
                        new_list.append(
                            mybir.InstNoOp(
                                name=f"{ins.name}-wsplit{j}",
                                engine=ins.engine,
                                sync_info=mybir.SyncInfo(on_wait=ch, on_update=[]),
                                bass_nofuse=True,
                            )
                        )
                    si.on_wait = chunks[-1]
                new_list.append(ins)
            blk.instructions[:] = new_list


def build_nc():
    nc = bass.Bass()
    emb_h = nc.declare_dram_parameter("emb", [BLOC, E, P], F32, isOutput=False)
    pred_h = nc.declare_dram_parameter("pred", [BLOC, L, P], F32, isOutput=False)
    lab_h = nc.declare_dram_parameter("lab", [BLOC, P], I32, isOutput=False)
    resm_h = nc.declare_dram_parameter("resm", [BLOC, MM_M, MM_N], F32, isOutput=True)
    ress_h = nc.declare_dram_parameter("ress", [BLOC, 128, NSC], F32, isOutput=True)

    with tile.TileContext(nc) as tc:
        with (
            tc.tile_pool(name="px", bufs=2) as px,           # f32 emb tiles
            tc.tile_pool(name="pxt", bufs=TGRP + 1) as pxt,  # transposed emb (bf16)
            tc.tile_pool(name="pxt2", bufs=2) as pxt2,       # squared transposed
            tc.tile_pool(name="ph1", bufs=2) as ph1,         # tree level outputs
            tc.tile_pool(name="ph2", bufs=2) as ph2,
            tc.tile_pool(name="ph3", bufs=2) as ph3,
            tc.tile_pool(name="ph4", bufs=2) as ph4,
            tc.tile_pool(name="pgi", bufs=2) as pgi,         # per-group inv chunks
            tc.tile_pool(name="pimg", bufs=1) as pimg,       # per-image label/weight
            tc.tile_pool(name="pce", bufs=1) as pce,         # CE pred tile
            tc.tile_pool(name="pcet", bufs=6) as pcet,       # CE temporaries
            tc.tile_pool(name="pres", bufs=2) as pres,
            tc.tile_pool(name="ppsum", bufs=2, space="PSUM") as ppsum,
        ):
            for img in range(BLOC):
                ress = pres.tile([128, NSC], F32, tag="ress")

                # ---- labels: load, cast, transpose to pixel-major ----
                lab_i = pimg.tile([128, PCOLS], I32, tag="lab_i")
                nc.sync.dma_start(lab_i[:], lab_h[img].rearrange("(q n) -> q n", q=128))
                lab_nb = pimg.tile([128, PCOLS], BF16, tag="lab_nb")
                nc.vector.tensor_copy(lab_nb[:], lab_i[:])
                lab32 = pimg.tile([128, PCOLS], BF16, tag="lab32")
                # col-permuted out AP: pixel g*PG + c*32 + p' lands at [g*32+p', c]
                nc.vector.transpose(
                    lab32[:].rearrange("p (r j) -> p j r", r=32), lab_nb[:]
                )

                # ---- weights w[:, c, m]: {oh1, oh2, oh1*inv, oh2*inv} ----
                # counts fall out of the accumulators of the is_equal builds
                w = pimg.tile([128, CIMG, 4], BF16, tag="w")
                nc.vector.tensor_scalar(
                    w[:, :, 0], lab32[:], 1.0, None, ALU.is_equal
                )
                nc.vector.tensor_scalar(
                    w[:, :, 1], lab32[:], 2.0, None, ALU.is_equal
                )
                # counts via accumulating copies (accum_out = sum(out))
                cnt1 = pimg.tile([128, CIMG], BF16, tag="cnt1")
                nc.vector.scalar_tensor_tensor(
                    cnt1[:], w[:, :, 0], 1.0, w[:, :, 0], ALU.bypass, ALU.mult,
                    accum_out=ress[:, 4:5],
                )
                cnt2 = pimg.tile([128, CIMG], BF16, tag="cnt2")
                nc.vector.scalar_tensor_tensor(
                    cnt2[:], w[:, :, 1], 1.0, w[:, :, 1], ALU.bypass, ALU.mult,
                    accum_out=ress[:, 5:6],
                )

                nrm2 = pimg.tile([128, CIMG], F32, tag="nrm2")
                acc = ppsum.tile([MM_M, MM_N], F32, tag="acc")

                for tg in range(NT // TGRP):  # 8 groups of 2 tiles
                    xts = []
                    for ti in range(TGRP):
                        t = tg * TGRP + ti
                        x = px.tile([128, FCOLS], F32, tag="x")
                        for g in range(G):
                            nc.sync.dma_start(
                                x[g * E : (g + 1) * E, :],
                                emb_h[
                                    img,
                                    :,
                                    g * PG + t * FCOLS : g * PG + (t + 1) * FCOLS,
                                ],
                            )
                        xb = px.tile([128, FCOLS], BF16, tag="xb")
                        nc.scalar.activation(xb[:], x[:], ACTF.Copy)
                        xt = pxt.tile([128, CB, 32], BF16, tag="xt")
                        nc.vector.transpose(xt[:], xb[:])
                        xts.append(xt)
                        # norm2 = sum of squares over the 32 channels:
                        # square (DVE STT, 4x mode) + halving-add tree
                        xt2 = pxt2.tile([128, CB, 32], BF16, tag="xt2")
                        nc.vector.scalar_tensor_tensor(
                            xt2[:], xt[:], 1.0, xt[:], ALU.bypass, ALU.mult
                        )
                        h1 = ph1.tile([128, CB, 16], BF16, tag="h1")
                        nc.vector.scalar_tensor_tensor(
                            h1[:], xt2[:, :, 0:16], 1.0, xt2[:, :, 16:32],
                            ALU.bypass, ALU.add,
                        )
                        h2 = ph2.tile([128, CB, 8], BF16, tag="h2")
                        nc.gpsimd.tensor_tensor(
                            h2[:], h1[:, :, 0:8], h1[:, :, 8:16], ALU.add
                        )
                        h3 = ph3.tile([128, CB, 4], BF16, tag="h3")
                        nc.gpsimd.tensor_tensor(
                            h3[:], h2[:, :, 0:4], h2[:, :, 4:8], ALU.add
                        )
                        h4 = ph4.tile([128, CB, 2], BF16, tag="h4")
                        nc.vector.scalar_tensor_tensor(
                            h4[:], h3[:, :, 0:2], 1.0, h3[:, :, 2:4],
                            ALU.bypass, ALU.add,
                        )
                        nc.vector.scalar_tensor_tensor(
                            nrm2[:, t * CB : (t + 1) * CB],
                            h4[:, :, 0], 1.0, h4[:, :, 1],
                            ALU.bypass, ALU.add,
                        )

                    # batched small ops over this group's c-range
                    gsl = slice(tg * TGRP * CB, (tg + 1) * TGRP * CB)
                    gn = TGRP * CB  # 256
                    # inv = 1/sqrt(nrm2) = exp(-0.5*ln(nrm2)); Ln/Exp share the
                    # act table with Square/Copy so no table reloads anywhere
                    lg = pgi.tile([128, gn], F32, tag="lg")
                    nc.scalar.activation(lg[:], nrm2[:, gsl], ACTF.Ln)
                    inv = pgi.tile([128, gn], F32, tag="inv")
                    nc.scalar.activation(inv[:], lg[:], ACTF.Exp, scale=-0.5)
                    nc.vector.scalar_tensor_tensor(
                        w[:, gsl, 2], w[:, gsl, 0], 1.0, inv[:],
                        ALU.bypass, ALU.mult,
                    )
                    nc.vector.scalar_tensor_tensor(
                        w[:, gsl, 3], w[:, gsl, 1], 1.0, inv[:],
                        ALU.bypass, ALU.mult,
                    )

                    for ti in range(TGRP):
                        t = tg * TGRP + ti
                        for mi in range(CB // CGRP):  # 16 matmuls per tile
                            c0 = t * CB + mi * CGRP
                            nc.tensor.matmul(
                                acc[:, :],
                                w[:, c0 : c0 + CGRP, :],
                                xts[ti][:, mi * CGRP : (mi + 1) * CGRP, :],
                                start=(t == 0 and mi == 0),
                                stop=(t == NT - 1 and mi == CB // CGRP - 1),
                            )

                # ---- cross-entropy partials ----
                p3 = pce.tile([128, L, PCOLS], F32, tag="p3")
                nc.sync.dma_start(
                    p3[:], pred_h[img].rearrange("c (q n) -> q c n", q=128)
                )
                e0 = pcet.tile([128, PCOLS], BF16, tag="cet")
                nc.scalar.activation(e0[:], p3[:, 0, :], ACTF.Exp)
                e1 = pcet.tile([128, PCOLS], BF16, tag="cet")
                nc.scalar.activation(e1[:], p3[:, 1, :], ACTF.Exp)
                e2 = pcet.tile([128, PCOLS], BF16, tag="cet")
                nc.scalar.activation(e2[:], p3[:, 2, :], ACTF.Exp)
                s01 = pcet.tile([128, PCOLS], BF16, tag="cet")
                nc.vector.scalar_tensor_tensor(
                    s01[:], e0[:], 1.0, e1[:], ALU.bypass, ALU.add
                )
                s012 = pcet.tile([128, PCOLS], BF16, tag="cet")
                nc.vector.scalar_tensor_tensor(
                    s012[:], s01[:], 1.0, e2[:], ALU.bypass, ALU.add
                )
                lntrash = pcet.tile([128, PCOLS], BF16, tag="cet")
                nc.scalar.activation(
                    lntrash[:], s012[:], ACTF.Ln, accum_out=ress[:, 0:1]
                )
                # picked = sum_c oh_c * pred_c, accumulated per class
                for c in range(L):
                    oh = pcet.tile([128, PCOLS], BF16, tag="cet")
                    nc.vector.tensor_scalar(
                        oh[:], lab_nb[:], float(c), None, ALU.is_equal
                    )
                    tr = pcet.tile([128, PCOLS], BF16, tag="cet")
                    nc.vector.scalar_tensor_tensor(
                        tr[:], p3[:, c, :], 1.0, oh[:], ALU.bypass, ALU.mult,
                        accum_out=ress[:, 1 + c : 2 + c],
                    )

                resm = pres.tile([MM_M, MM_N], F32, tag="resm")
                nc.vector.tensor_copy(resm[:], acc[:])
                nc.sync.dma_start(resm_h[img], resm[:])
                nc.sync.dma_start(ress_h[img], ress[:])

    _split_oversized_waits(nc)
    return nc


_NC_CACHE = None


def _get_nc():
    global _NC_CACHE
    if _NC_CACHE is None:
        _NC_CACHE = build_nc()
    return _NC_CACHE


def _host_epilogue(resm, ress, neighbor):
    """resm: (MM_M, MM_N) f32; ress: (128, NSC) f32 partials for one image."""
    A = resm.astype(np.float64)
    rs = ress.astype(np.float64)
    M4 = np.zeros((4, 32))
    for cp in range(CGRP):
        M4 += A[cp * 4 : (cp + 1) * 4, cp * 32 : (cp + 1) * 32]
    t1, t2, s1, s2 = M4[0], M4[1], M4[2], M4[3]
    c1, c2 = rs[:, 4].sum(), rs[:, 5].sum()

    lse_sum = rs[:, 0].sum()
    picked_sum = rs[:, 1].sum() + rs[:, 2].sum() + rs[:, 3].sum()
    ce = (lse_sum - picked_sum) / P

    m1, m2 = t1 / c1, t2 / c2
    nm1 = m1 / max(np.linalg.norm(m1), 1e-12)
    nm2 = m2 / max(np.linalg.norm(m2), 1e-12)
    intra = ((1.0 - nm1 @ s1 / c1) + (1.0 - nm2 @ s2 / c2)) / (L - 1)

    nm = np.zeros((L, E))
    nm[1], nm[2] = nm1, nm2
    S = nm @ nm.T
    nb = neighbor.astype(np.int64)
    valid = np.cumprod((nb != 0).astype(np.float64), axis=1)
    rows = np.broadcast_to(np.arange(L)[:, None], nb.shape)
    row_ok = (rows >= 1).astype(np.float64)
    mask = np.zeros((L, L))
    np.maximum.at(mask, (rows.ravel(), nb.ravel()), (valid * row_ok).ravel())
    inter = (S * mask).sum() / mask.sum()

    return intra + inter + ce


def kernel(embedding, prediction, class_label, neighbor):
    embedding = np.ascontiguousarray(np.asarray(embedding), dtype=np.float32)
    prediction = np.ascontiguousarray(np.asarray(prediction), dtype=np.float32)
    class_label = np.ascontiguousarray(np.asarray(class_label), dtype=np.int32)
    neighbor = np.asarray(neighbor)

    nc = _get_nc()
    in_maps = []
    for core in range(NCORES):
        sl = slice(core * BLOC, (core + 1) * BLOC)
        in_maps.append(
            {
                "emb": embedding[sl].reshape(BLOC, E, P),
                "pred": prediction[sl].reshape(BLOC, L, P),
                "lab": class_label[sl].reshape(BLOC, P),
            }
        )
    out = run_bass_kernel_spmd(nc, in_maps, core_ids=list(range(NCORES)))

    total = 0.0
    for core in range(NCORES):
        for i in range(BLOC):
            b = core * BLOC + i
            total += _host_epilogue(
                out.results[core]["resm"][i], out.results[core]["ress"][i],
                neighbor[b],
            )
    return np.float32(total)


# revision 15
# speedup vs baseline: 1.3439x; 1.3439x over previous
"""Trainium2 Bass kernel for nn_Criterion_74448963109285 (segment_reduce criterion).

Strategy (pure data parallel, 2 images per core on 8 cores):
  Per image the loss is  intra + inter + ce  where every term reduces to a
  handful of tiny quantities:
    - segment sums over pixels per label l in {1,2}:
        t_l[e] = sum_{p: lab=l} emb[e,p]          (raw)
        s_l[e] = sum_{p: lab=l} emb[e,p]/||emb_p|| (normalized)
        c_l    = count of pixels with label l
    - ce partials: sum_p logsumexp(pred[:,p]) and sum_p pred[lab_p, p]
  The device computes only these reductions; the final scalar math runs on
  host in float64.

  Engine plan (per core = 2 images, 32 tiles of [128, 4096]):
    DMA    one 2MB dma_start per tile (HWDGE, alternating sync/scalar rings)
    DVE    stream-transpose f32->bf16 (cast fused into the transpose),
           level-1 halving add of the squares (scalar_tensor_tensor, 4x mode),
           CE picked-logit accumulation, label/weight prep
    Scalar SQUARE activation on the transposed tile, Rsqrt for 1/||x||,
           CE exp/ln (+ row-accumulated logsumexp)
    GpSimd levels 2-5 of the norm2 halving-add tree
    PE     block-diagonal matmuls accumulating {oh1,oh2,oh1/|x|,oh2/|x|} x
           {32 channels} segment sums into one PSUM tile
  Counts come for free from accum_out on the is_equal weight builds, so the
  matmul rhs has no ones column (stride-32, fully aligned).
"""

import numpy as np

import concourse.bass as bass
import concourse.tile as tile
from concourse import mybir
from concourse.bass_utils import run_bass_kernel_spmd

F32 = mybir.dt.float32
BF16 = mybir.dt.bfloat16
I32 = mybir.dt.int32
ALU = mybir.AluOpType
ACTF = mybir.ActivationFunctionType

B, E, H, W, L = 16, 32, 512, 512, 3
P = H * W                  # 262144 pixels per image
NCORES = 8
BLOC = B // NCORES         # 2 images per core
G = 4                      # pixel groups packed into partitions (4*32ch=128)
PG = P // G                # 65536 pixels per group
NT = 16                    # tiles per image
FCOLS = PG // NT           # 4096 pixel columns per tile (per group)
CB = FCOLS // 32           # 128 c-blocks (32 px each) per tile
CIMG = PG // 32            # 2048 c-blocks per image
TGRP = 2                   # tiles per rsqrt/weight batch group
CGRP = 8                   # c-blocks per matmul (N = 8*32 = 256)
MM_M = 4 * CGRP            # 32 output partitions
MM_N = 32 * CGRP           # 256 output cols
PCOLS = P // 128           # 2048 label/pred columns per image
NSC = 8                    # small-result columns


def _split_oversized_waits(nc, max_waits=1):
    """This walrus build accepts only one sync wait per instruction; move
    extra waits onto single-wait NOPs preceding the instruction."""
    for fn in nc.m.functions:
        for blk in fn.blocks:
            new_list = []
            for ins in blk.instructions:
                si = getattr(ins, "sync_info", None)
                if si is not None and si.on_wait and len(si.on_wait) > max_waits:
                    waits = list(si.on_wait)
                    chunks = [
                        waits[i : i + max_waits]
                        for i in range(0, len(waits), max_waits)
                    ]
                    for j, ch in enumerate(chunks[:-1]):
                        new_list.append(
                            mybir.InstNoOp(
                                name=f"{ins.name}-wsplit{j}",
                                engine=ins.engine,
                                sync_info=mybir.SyncInfo(on_wait=ch, on_update=[]),
                                bass_nofuse=True,
                            )
                        )
                    si.on_wait = chunks[-1]
                new_list.append(ins)
            blk.instructions[:] = new_list


def build_nc():
    nc = bass.Bass()
    emb_h = nc.declare_dram_parameter("emb", [BLOC, E, P], F32, isOutput=False)
    pred_h = nc.declare_dram_parameter("pred", [BLOC, L, P], F32, isOutput=False)
    lab_h = nc.declare_dram_parameter("lab", [BLOC, P], I32, isOutput=False)
    resm_h = nc.declare_dram_parameter("resm", [BLOC, MM_M, MM_N], F32, isOutput=True)
    ress_h = nc.declare_dram_parameter("ress", [BLOC, 128, NSC], F32, isOutput=True)

    with tile.TileContext(nc) as tc:
        with (
            tc.tile_pool(name="px", bufs=2) as px,           # f32 emb tiles
            tc.tile_pool(name="pxt", bufs=TGRP + 1) as pxt,  # transposed emb (bf16)
            tc.tile_pool(name="pxt2", bufs=2) as pxt2,       # squared transposed
            tc.tile_pool(name="ph1", bufs=2) as ph1,         # tree level outputs
            tc.tile_pool(name="ph2", bufs=2) as ph2,
            tc.tile_pool(name="ph3", bufs=2) as ph3,
            tc.tile_pool(name="ph4", bufs=2) as ph4,
            tc.tile_pool(name="pgi", bufs=2) as pgi,         # per-group inv chunks
            tc.tile_pool(name="pimg", bufs=1) as pimg,       # per-image label/weight
            tc.tile_pool(name="pce", bufs=1) as pce,         # CE pred tile
            tc.tile_pool(name="pcet", bufs=6) as pcet,       # CE temporaries
            tc.tile_pool(name="pres", bufs=2) as pres,
            tc.tile_pool(name="ppsum", bufs=2, space="PSUM") as ppsum,
        ):
            for img in range(BLOC):
                ress = pres.tile([128, NSC], F32, tag="ress")

                # ---- labels: load, cast, transpose to pixel-major ----
                lab_i = pimg.tile([128, PCOLS], I32, tag="lab_i")
                nc.sync.dma_start(lab_i[:], lab_h[img].rearrange("(q n) -> q n", q=128))
                lab_nb = pimg.tile([128, PCOLS], BF16, tag="lab_nb")
                nc.vector.tensor_copy(lab_nb[:], lab_i[:])
                lab32 = pimg.tile([128, PCOLS], BF16, tag="lab32")
                # col-permuted out AP: pixel g*PG + c*32 + p' lands at [g*32+p', c]
                nc.vector.transpose(
                    lab32[:].rearrange("p (r j) -> p j r", r=32), lab_nb[:]
                )

                # ---- weights w[:, c, m]: {oh1, oh2, oh1*inv, oh2*inv} ----
                # counts fall out of the accumulators of the is_equal builds
                w = pimg.tile([128, CIMG, 4], BF16, tag="w")
                nc.vector.tensor_scalar(
                    w[:, :, 0], lab32[:], 1.0, None, ALU.is_equal
                )
                nc.vector.tensor_scalar(
                    w[:, :, 1], lab32[:], 2.0, None, ALU.is_equal
                )
                # counts via accumulating copies (accum_out = sum(out))
                cnt1 = pimg.tile([128, CIMG], BF16, tag="cnt1")
                nc.vector.scalar_tensor_tensor(
                    cnt1[:], w[:, :, 0], 1.0, w[:, :, 0], ALU.bypass, ALU.mult,
                    accum_out=ress[:, 4:5],
                )
                cnt2 = pimg.tile([128, CIMG], BF16, tag="cnt2")
                nc.vector.scalar_tensor_tensor(
                    cnt2[:], w[:, :, 1], 1.0, w[:, :, 1], ALU.bypass, ALU.mult,
                    accum_out=ress[:, 5:6],
                )

                nrm2 = pimg.tile([128, CIMG], F32, tag="nrm2")
                acc = ppsum.tile([MM_M, MM_N], F32, tag="acc")

                for tg in range(NT // TGRP):  # 8 groups of 2 tiles
                    xts = []
                    for ti in range(TGRP):
                        t = tg * TGRP + ti
                        x = px.tile([128, FCOLS], F32, tag="x")
                        for g in range(G):
                            nc.sync.dma_start(
                                x[g * E : (g + 1) * E, :],
                                emb_h[
                                    img,
                                    :,
                                    g * PG + t * FCOLS : g * PG + (t + 1) * FCOLS,
                                ],
                            )
                        xb = px.tile([128, FCOLS], BF16, tag="xb")
                        nc.scalar.activation(xb[:], x[:], ACTF.Copy)
                        xt = pxt.tile([128, CB, 32], BF16, tag="xt")
                        nc.vector.transpose(xt[:], xb[:])
                        xts.append(xt)
                        # norm2 = sum of squares over the 32 channels:
                        # square (DVE TT mult, 2x) + halving-add tree (gpsimd)
                        xt2 = pxt2.tile([128, CB, 32], BF16, tag="xt2")
                        nc.vector.tensor_tensor(xt2[:], xt[:], xt[:], ALU.mult)
                        h1 = ph1.tile([128, CB, 16], BF16, tag="h1")
                        nc.vector.tensor_tensor(
                            h1[:], xt2[:, :, 0:16], xt2[:, :, 16:32], ALU.add
                        )
                        h2 = ph2.tile([128, CB, 8], BF16, tag="h2")
                        nc.gpsimd.tensor_tensor(
                            h2[:], h1[:, :, 0:8], h1[:, :, 8:16], ALU.add
                        )
                        h3 = ph3.tile([128, CB, 4], BF16, tag="h3")
                        nc.gpsimd.tensor_tensor(
                            h3[:], h2[:, :, 0:4], h2[:, :, 4:8], ALU.add
                        )
                        h4 = ph4.tile([128, CB, 2], BF16, tag="h4")
                        nc.gpsimd.tensor_tensor(
                            h4[:], h3[:, :, 0:2], h3[:, :, 2:4], ALU.add
                        )
                        nc.gpsimd.tensor_tensor(
                            nrm2[:, t * CB : (t + 1) * CB],
                            h4[:, :, 0], h4[:, :, 1], ALU.add,
                        )

                    # batched small ops over this group's c-range
                    gsl = slice(tg * TGRP * CB, (tg + 1) * TGRP * CB)
                    gn = TGRP * CB  # 256
                    # inv = 1/sqrt(nrm2) = exp(-0.5*ln(nrm2)); Ln/Exp share the
                    # act table with Square/Copy so no table reloads anywhere
                    lg = pgi.tile([128, gn], F32, tag="lg")
                    nc.scalar.activation(lg[:], nrm2[:, gsl], ACTF.Ln)
                    inv = pgi.tile([128, gn], BF16, tag="inv")
                    nc.scalar.activation(inv[:], lg[:], ACTF.Exp, scale=-0.5)
                    nc.vector.tensor_tensor(
                        w[:, gsl, 2], w[:, gsl, 0], inv[:], ALU.mult
                    )
                    nc.vector.tensor_tensor(
                        w[:, gsl, 3], w[:, gsl, 1], inv[:], ALU.mult
                    )

                    for ti in range(TGRP):
                        t = tg * TGRP + ti
                        for mi in range(CB // CGRP):  # 16 matmuls per tile
                            c0 = t * CB + mi * CGRP
                            nc.tensor.matmul(
                                acc[:, :],
                                w[:, c0 : c0 + CGRP, :],
                                xts[ti][:, mi * CGRP : (mi + 1) * CGRP, :],
                                start=(t == 0 and mi == 0),
                                stop=(t == NT - 1 and mi == CB // CGRP - 1),
                            )

                # ---- cross-entropy partials ----
                p3 = pce.tile([128, L, PCOLS], F32, tag="p3")
                nc.sync.dma_start(
                    p3[:], pred_h[img].rearrange("c (q n) -> q c n", q=128)
                )
                e0 = pcet.tile([128, PCOLS], BF16, tag="cet")
                nc.scalar.activation(e0[:], p3[:, 0, :], ACTF.Exp)
                e1 = pcet.tile([128, PCOLS], BF16, tag="cet")
                nc.scalar.activation(e1[:], p3[:, 1, :], ACTF.Exp)
                e2 = pcet.tile([128, PCOLS], BF16, tag="cet")
                nc.scalar.activation(e2[:], p3[:, 2, :], ACTF.Exp)
                s01 = pcet.tile([128, PCOLS], BF16, tag="cet")
                nc.vector.tensor_tensor(s01[:], e0[:], e1[:], ALU.add)
                s012 = pcet.tile([128, PCOLS], BF16, tag="cet")
                nc.vector.tensor_tensor(s012[:], s01[:], e2[:], ALU.add)
                lntrash = pcet.tile([128, PCOLS], BF16, tag="cet")
                nc.scalar.activation(
                    lntrash[:], s012[:], ACTF.Ln, accum_out=ress[:, 0:1]
                )
                # picked = sum_c oh_c * pred_c, accumulated per class
                for c in range(L):
                    oh = pcet.tile([128, PCOLS], BF16, tag="cet")
                    nc.vector.tensor_scalar(
                        oh[:], lab_nb[:], float(c), None, ALU.is_equal
                    )
                    tr = pcet.tile([128, PCOLS], BF16, tag="cet")
                    nc.vector.scalar_tensor_tensor(
                        tr[:], p3[:, c, :], 1.0, oh[:], ALU.bypass, ALU.mult,
                        accum_out=ress[:, 1 + c : 2 + c],
                    )

                resm = pres.tile([MM_M, MM_N], F32, tag="resm")
                nc.vector.tensor_copy(resm[:], acc[:])
                nc.sync.dma_start(resm_h[img], resm[:])
                nc.sync.dma_start(ress_h[img], ress[:])

    _split_oversized_waits(nc)
    return nc


_NC_CACHE = None


def _get_nc():
    global _NC_CACHE
    if _NC_CACHE is None:
        _NC_CACHE = build_nc()
    return _NC_CACHE


def _host_epilogue(resm, ress, neighbor):
    """resm: (MM_M, MM_N) f32; ress: (128, NSC) f32 partials for one image."""
    A = resm.astype(np.float64)
    rs = ress.astype(np.float64)
    M4 = np.zeros((4, 32))
    for cp in range(CGRP):
        M4 += A[cp * 4 : (cp + 1) * 4, cp * 32 : (cp + 1) * 32]
    t1, t2, s1, s2 = M4[0], M4[1], M4[2], M4[3]
    c1, c2 = rs[:, 4].sum(), rs[:, 5].sum()

    lse_sum = rs[:, 0].sum()
    picked_sum = rs[:, 1].sum() + rs[:, 2].sum() + rs[:, 3].sum()
    ce = (lse_sum - picked_sum) / P

    m1, m2 = t1 / c1, t2 / c2
    nm1 = m1 / max(np.linalg.norm(m1), 1e-12)
    nm2 = m2 / max(np.linalg.norm(m2), 1e-12)
    intra = ((1.0 - nm1 @ s1 / c1) + (1.0 - nm2 @ s2 / c2)) / (L - 1)

    nm = np.zeros((L, E))
    nm[1], nm[2] = nm1, nm2
    S = nm @ nm.T
    nb = neighbor.astype(np.int64)
    valid = np.cumprod((nb != 0).astype(np.float64), axis=1)
    rows = np.broadcast_to(np.arange(L)[:, None], nb.shape)
    row_ok = (rows >= 1).astype(np.float64)
    mask = np.zeros((L, L))
    np.maximum.at(mask, (rows.ravel(), nb.ravel()), (valid * row_ok).ravel())
    inter = (S * mask).sum() / mask.sum()

    return intra + inter + ce


def kernel(embedding, prediction, class_label, neighbor):
    embedding = np.ascontiguousarray(np.asarray(embedding), dtype=np.float32)
    prediction = np.ascontiguousarray(np.asarray(prediction), dtype=np.float32)
    class_label = np.ascontiguousarray(np.asarray(class_label), dtype=np.int32)
    neighbor = np.asarray(neighbor)

    nc = _get_nc()
    in_maps = []
    for core in range(NCORES):
        sl = slice(core * BLOC, (core + 1) * BLOC)
        in_maps.append(
            {
                "emb": embedding[sl].reshape(BLOC, E, P),
                "pred": prediction[sl].reshape(BLOC, L, P),
                "lab": class_label[sl].reshape(BLOC, P),
            }
        )
    out = run_bass_kernel_spmd(nc, in_maps, core_ids=list(range(NCORES)))

    total = 0.0
    for core in range(NCORES):
        for i in range(BLOC):
            b = core * BLOC + i
            total += _host_epilogue(
                out.results[core]["resm"][i], out.results[core]["ress"][i],
                neighbor[b],
            )
    return np.float32(total)
